# revision 12
# baseline (speedup 1.0000x reference)
"""Trainium2 Bass kernel for nn_CustomConv1D_d (rank-1 dense conv1d, stride 21).

Math: out[b, t, o] = r[b, t] for all o in [0, 237), where
  r[b, t] = sum_k w[k] * sum_c x[b, 21 t + k, c],  w = softmax(p3*i + p4*i^2).

Strategy (pure data parallel over batch, 4 batches per core):
  - Per core, view x as flat [43008, 237]; each output group t owns 21
    consecutive rows = 19908 contiguous bytes. Load tiles [128 groups,
    21*237] — one fully-contiguous 19908B DMA descriptor per partition.
    Deep xin pool (7 bufs) keeps the 16 SDMA queues fed; the stream
    sustains ~400 GB/s when issue never stalls.
  - r[g] = dot(x_group, w repeated per channel): fused DVE
    scalar_tensor_tensor per tap-chunk (product + free-dim accumulate,
    ~1.03 cyc/elem) with a stride-0 broadcast view of w as in1 — no
    separate weight tile, no segmented reduce.
  - A few tiles compute on the ACT engine instead (per-tap
    activation(scale=w[k]) with free-dim accumulate) so the DVE stays
    off the critical path; the last tile runs on ACT so its final taps
    finish ~1us after their DMA lands.
  - HWDGE deals a DMA's per-partition descriptors over gcd(P, 16)
    engines in contiguous runs; engine 15 is ~14% slower than peers, so
    one tile loads as [0:120)+[120:128) (both halves deal 8-way onto
    engines 0-7), shifting 1/16 of the stream off the slow engine.
  - ACT broadcasts each r column across 237 channels; ACT-issued DMAs
    stream results out without stalling the input ring.
"""

import numpy as np
from contextlib import ExitStack

import concourse.bass as bass
import concourse.tile as tile
import concourse.mybir as mybir
from concourse.bass_utils import run_bass_kernel_spmd

TAPS = 21
C = 237
B = 32
L = 10752
T = 512
NCORES = 8
BPC = B // NCORES            # 4 batches per core
ROWS = BPC * L               # 43008 rows per core
GROUPS = BPC * T             # 2048 groups per core
NQ = GROUPS // 128           # 16 tiles of 128 groups
GROUP_ROWS = 128 * TAPS      # 2688 input rows per tile
FD = TAPS * C                # 4977 elements per group
F32 = mybir.dt.float32

# Tap-chunk splits: early tiles ramp the DVE pipeline, late tiles keep
# the post-DMA serial tail short.
SPLITS = {0: [11, 10], 1: [11, 10], NQ - 2: [11, 10], NQ - 1: [10, 8, 3]}
# Tiles computed on the ACT engine: none — ACT's per-tap accumulate
# costs ~860ns/tap (ACTIVATE + READ_ACCUMULATOR), far too slow, and
# holding xin buffers stalls DMA issue.
ACT_TILES = set()
# Optional straggler mitigation: a tile in this set loads as
# [0:SKEW_P)+[SKEW_P:128), dealing 8-way onto engines 0-7 only. At the
# observed ~430 GB/s the stream is engine-bound and the imbalance costs
# more than slow engine 15 does — keep empty.
SKEW = set()
SKEW_P = 120
# Output blocks: ACT broadcast + one store per block.
OUT_GROUPS = [[0, 1, 2, 3], [4, 5, 6, 7], [8, 9, 10, 11], [12, 13], [14], [15]]


class _TileContext(tile.TileContext):
    """TileContext with a post-scheduling pass that splits instructions
    carrying >1 sem wait onto preceding single-wait nops on the same
    engine — the pinned neuronxcc rejects instructions with multiple
    sync wait commands."""

    def schedule_and_allocate(self):
        ret = super().schedule_and_allocate()
        self._split_multi_waits()
        return ret

    def _split_multi_waits(self):
        nc = self.nc
        for fn in nc.m.functions:
            for bb in fn.blocks:
                if not any(
                    inst.sync_info
                    and inst.sync_info.on_wait
                    and len(inst.sync_info.on_wait) > 1
                    for inst in bb.instructions
                ):
                    continue
                new_insts = []
                for inst in bb.instructions:
                    si = inst.sync_info
                    waits = list(si.on_wait) if si and si.on_wait else []
                    if len(waits) > 1:
                        si.on_wait = waits[-1:]
                        for w in waits[:-1]:
                            nop = mybir.InstNoOp(
                                name=f"I-splitw-{nc.next_id()}",
                                engine=inst.engine,
                                sync_info=mybir.SyncInfo(on_wait=[w], on_update=[]),
                            )
                            nc.register_instruction(nop, overwrite=True)
                            new_insts.append(nop)
                    new_insts.append(inst)
                bb.instructions[:] = new_insts


def _build():
    nc = bass.Bass("TRN2", target_bir_lowering=False, debug=False)
    x = nc.dram_tensor("x", [ROWS, C], F32, kind="ExternalInput").ap()
    wv = nc.dram_tensor("wv", [128, TAPS], F32, kind="ExternalInput").ap()
    y = nc.dram_tensor("y", [GROUPS, C], F32, kind="ExternalOutput").ap()

    # partial-accumulator column layout
    pcols = {}
    npart = 0
    for q in range(NQ):
        if q in ACT_TILES:
            pcols[q] = npart
            npart += TAPS
        elif len(SPLITS.get(q, [TAPS])) > 1:
            pcols[q] = npart
            npart += len(SPLITS[q])

    with _TileContext(nc) as tc:
        with ExitStack() as ctx:
            xin = ctx.enter_context(tc.tile_pool(name="xin", bufs=8))
            sp = ctx.enter_context(tc.tile_pool(name="sp", bufs=1))
            op = ctx.enter_context(tc.tile_pool(name="op", bufs=2))

            wvt = sp.tile([128, TAPS], F32)
            nc.sync.dma_start(wvt[:], wv[:, :])

            racc = sp.tile([128, NQ], F32)
            pacc = sp.tile([128, npart], F32)
            scr = sp.tile([128, FD], F32)
            scra = sp.tile([128, C], F32)

            for qs in OUT_GROUPS:
                for q in qs:
                    xt = xin.tile([128, FD], F32, tag="xt")
                    v3 = xt.rearrange("p (k c) -> p k c", c=C)
                    src = x[q * GROUP_ROWS : (q + 1) * GROUP_ROWS, :].rearrange(
                        "(p k) c -> p k c", k=TAPS
                    )
                    chunks = SPLITS.get(q, [TAPS])
                    k0 = 0
                    for ci, tk in enumerate(chunks):
                        if q in SKEW:
                            nc.sync.dma_start(
                                v3[:SKEW_P, k0 : k0 + tk, :],
                                src[:SKEW_P, k0 : k0 + tk, :],
                            )
                            nc.sync.dma_start(
                                v3[SKEW_P:, k0 : k0 + tk, :],
                                src[SKEW_P:, k0 : k0 + tk, :],
                            )
                        else:
                            nc.sync.dma_start(
                                v3[:, k0 : k0 + tk, :],
                                src[:, k0 : k0 + tk, :],
                            )
                        if q in ACT_TILES:
                            # per-tap: pacc[:, col+k] = w[k] * sum_c x[p,k,c]
                            for k in range(k0, k0 + tk):
                                nc.scalar.activation(
                                    scra[:],
                                    v3[:, k, :],
                                    mybir.ActivationFunctionType.Identity,
                                    scale=wvt[:, k : k + 1],
                                    accum_out=pacc[:, pcols[q] + k : pcols[q] + k + 1],
                                )
                        else:
                            acc_col = (
                                racc[:, q : q + 1]
                                if len(chunks) == 1
                                else pacc[:, pcols[q] + ci : pcols[q] + ci + 1]
                            )
                            nc.vector.scalar_tensor_tensor(
                                out=scr.rearrange("p (k c) -> p k c", c=C)[:, :tk, :],
                                in0=v3[:, k0 : k0 + tk, :],
                                scalar=1.0,
                                in1=wvt[:, k0 : k0 + tk, None].broadcast_to(
                                    [128, tk, C]
                                ),
                                op0=mybir.AluOpType.mult,
                                op1=mybir.AluOpType.mult,
                                accum_out=acc_col,
                            )
                        k0 += tk
                    if q in ACT_TILES:
                        # ACT self-combine: racc[:, q] = sum_k pacc cols
                        nc.scalar.activation(
                            scra[:, :TAPS],
                            pacc[:, pcols[q] : pcols[q] + TAPS],
                            mybir.ActivationFunctionType.Identity,
                            accum_out=racc[:, q : q + 1],
                        )
                    elif len(chunks) > 1:
                        nc.vector.reduce_sum(
                            racc[:, q : q + 1],
                            pacc[:, pcols[q] : pcols[q] + len(chunks)].rearrange(
                                "p (o k) -> p o k", o=1
                            ),
                            axis=mybir.AxisListType.X,
                        )

                nb = len(qs)
                osb = op.tile([128, 4 * C], F32, tag="osb")
                for j, qg in enumerate(qs):
                    if qs[0] == NQ - 1:
                        # final block: broadcast on the DVE right after its
                        # combine — skips an ACT hop on the critical tail
                        nc.vector.tensor_copy(
                            osb[:, j * C : (j + 1) * C],
                            racc[:, qg : qg + 1].broadcast_to([128, C]),
                        )
                    else:
                        nc.scalar.activation(
                            osb[:, j * C : (j + 1) * C],
                            racc[:, qg : qg + 1].broadcast_to([128, C]),
                            mybir.ActivationFunctionType.Identity,
                        )
                nc.scalar.dma_start(
                    y[qs[0] * 128 : (qs[-1] + 1) * 128, :].rearrange(
                        "(q p) c -> p q c", p=128
                    ),
                    osb[:, 0 : nb * C].rearrange("p (q c) -> p q c", c=C),
                )
    return nc


_NC_CACHE = {}


def _get_nc():
    if "nc" not in _NC_CACHE:
        _NC_CACHE["nc"] = _build()
    return _NC_CACHE["nc"]


def _tap_weights(param3: float, param4: float) -> np.ndarray:
    i = np.arange(1, TAPS + 1, dtype=np.float32)
    logits = (np.float32(param3) * i + np.float32(param4) * i * i).astype(np.float32)
    e = np.exp(logits - logits.max(), dtype=np.float32)
    return (e / e.sum()).astype(np.float32)


def run_with_results(inputs, **spmd_kwargs):
    x = np.ascontiguousarray(np.asarray(inputs["inputs"], dtype=np.float32))
    assert x.shape == (B, L, C), x.shape
    w = _tap_weights(
        float(np.asarray(inputs["param3"])), float(np.asarray(inputs["param4"]))
    )
    wv = np.ascontiguousarray(np.broadcast_to(w, (128, TAPS)))
    xs = x.reshape(NCORES, ROWS, C)
    in_maps = [{"x": xs[i], "wv": wv} for i in range(NCORES)]
    res = run_bass_kernel_spmd(_get_nc(), in_maps, list(range(NCORES)), **spmd_kwargs)
    out = np.stack([res.results[i]["y"] for i in range(NCORES)])
    return out.reshape(B, T, C).astype(np.float32, copy=False), res


def kernel(**inputs) -> np.ndarray:
    out, _ = run_with_results(inputs)
    return out
